# revision 3
# baseline (speedup 1.0000x reference)
"""Trainium2 Bass kernel for nn_MinibatchDiscrimination (B=256, F=1024, O=128, K=8).

out = concat([x, c]),  c[i,o] = sum_{j!=i} exp(-sum_k |M[j,o,k]-M[i,o,k]|),
M = x @ T.

Identity: |a-b| = 2*max(a,b) - a - b, so with S = sum_k M and H = sum_k max:
  exp(-diffs) = exp(-2*H + S_i + S_j).

Layout: partition p = o (all 128 output features), k is the group dim.
  mt [128, (k 8, j 160)] bf16 from an fp8 DoubleRow GEMM (k-major W image).
  tt-max per k: d[k][o, delta*32+i] = max(mt[o, k*160+1+delta+i], mt[o, k*160+i])
    - one batched DVE tensor_tensor(max) per k over all 32 rows x 128 window,
      (delta,i) APs with i innermost (unit stride) so DVE 2x mode engages.
  ksum: PSUM banks tile the DELTA axis (bank b = delta in [16b,16b+16) x all i),
    so every matmul rhs is a flat contiguous 512-col slice of d[k]; the
    "mask" is a 128x128 identity so matmuls are full-width PSUM accumulates.
  S: identity-matmul accumulation over the 8 k-blocks of mt; sneg = -S/2
    (ScalarE); ssum[o, delta*32+i] = -(S_i + S_j)/2 via one batched DVE add
    with the same (delta,i) window APs; one seed matmul per bank closes the
    accumulation group.
  exp per bank: flat [128,512] PSUM -> band slice (band col = delta*32+i),
    scale=-2, bias=0, fully contiguous; band DMA'd out in 4 chunks.

Distribution: c rows sharded across 8 cores (32 each) via host-side column
rotation of x^T; every core runs the full GEMM redundantly (no collectives).
Host assembles row-part + shifted column-part and concats with x.
"""

import numpy as np
import ml_dtypes

B, F, O, K = 256, 1024, 128, 8
NCORES = 8
IB = B // NCORES  # 32 rows per core
WIN = 128
JW = 160  # local j extent
FC = F // 128

_cache = {}


def _build():
    from contextlib import ExitStack
    import concourse.bacc as bacc
    import concourse.tile as tile
    import concourse.mybir as mybir
    from concourse.bass import AP

    dt = mybir.dt
    Alu = mybir.AluOpType
    Act = mybir.ActivationFunctionType
    DR = mybir.MatmulPerfMode.DoubleRow

    nc = bacc.Bacc(
        "TRN2", target_bir_lowering=False, debug=False, enable_asserts=False
    )
    w0 = nc.dram_tensor("w0", (128, 4 * FC * 128), dt.float8e4, kind="ExternalInput").ap()
    w1 = nc.dram_tensor("w1", (128, 4 * FC * 128), dt.float8e4, kind="ExternalInput").ap()
    xtd = nc.dram_tensor("xt", (128, FC * JW), dt.float8e4, kind="ExternalInput").ap()
    idd = nc.dram_tensor("idm", (128, 128), dt.bfloat16, kind="ExternalInput").ap()
    eout = nc.dram_tensor("eb", (O, WIN * IB), dt.bfloat16, kind="ExternalOutput").ap()

    NB = 8  # delta-block PSUM banks
    NBA = 6  # banks coexisting with the GEMM pool

    with ExitStack() as ctx:
        tc = ctx.enter_context(tile.TileContext(nc))
        inpool = ctx.enter_context(tc.tile_pool(name="inp", bufs=1))
        mpool = ctx.enter_context(tc.tile_pool(name="mt", bufs=1))
        dpool = ctx.enter_context(tc.tile_pool(name="d", bufs=1))
        cpool = ctx.enter_context(tc.tile_pool(name="c", bufs=1))

        w_sb = inpool.tile([128, 8 * FC * 128], dt.float8e4, tag="wsb")
        x_sb = inpool.tile([128, FC * JW], dt.float8e4, tag="xsb")
        id_sb = inpool.tile([128, 128], dt.bfloat16, tag="idm")
        nc.sync.dma_start(w_sb[:, 0 : 4 * FC * 128], w0)
        nc.gpsimd.dma_start(w_sb[:, 4 * FC * 128 :], w1)
        nc.scalar.dma_start(x_sb[:], xtd)
        nc.scalar.dma_start(id_sb[:], idd)

        mt = mpool.tile([128, 8 * JW], dt.bfloat16, tag="mt")
        sneg = mpool.tile([128, JW], dt.bfloat16, tag="sneg")
        ssum = mpool.tile([128, WIN * IB], dt.bfloat16, tag="ssum")
        d = [
            dpool.tile([128, WIN * IB], dt.bfloat16, tag=f"d{k}", name=f"d{k}")
            for k in range(K)
        ]
        band = cpool.tile([128, WIN * IB], dt.bfloat16, tag="band")

        def win_ap(tile_ap, base_off, sd, si):
            """[p, (delta: 128 x stride sd, i: 32 x stride si)], i innermost."""
            prow = list(tile_ap.ap[0])
            return AP(tile_ap.tensor, base_off, [prow, [sd, WIN], [si, IB]])

        with tc.tile_pool(name="kpsA", bufs=NBA, space="PSUM") as kpsA:
            pts = {}
            for b in range(NBA):
                pts[b] = kpsA.tile([128, 512], dt.float32, tag="pt", name=f"pt{b}")

            def layer(k, banks):
                for b in banks:
                    nc.tensor.matmul(
                        pts[b][:],
                        id_sb[:],
                        d[k][:, b * 512 : (b + 1) * 512],
                        start=(k == 0),
                        stop=False,
                        skip_group_check=True,
                    )

            with tc.tile_pool(name="gps", bufs=2, space="PSUM") as gps:
                for k in range(K):
                    gm = gps.tile([128, JW], dt.float32, tag="gm", name=f"gm{k}")
                    for pr in range(FC // 2):
                        base = k * FC * 128 + pr * 256
                        nc.tensor.matmul(
                            gm[:],
                            w_sb[:, base : base + 256].rearrange(
                                "p (two m) -> p two m", two=2
                            ),
                            x_sb[:, pr * 2 * JW : (pr + 1) * 2 * JW].rearrange(
                                "p (two n) -> p two n", two=2
                            ),
                            start=(pr == 0),
                            stop=(pr == FC // 2 - 1),
                            perf_mode=DR,
                        )
                    nc.scalar.copy(mt[:, k * JW : (k + 1) * JW], gm[:])
                    if k < K - 2:
                        nc.vector.tensor_tensor(
                            d[k][:].rearrange("p (dd i) -> p dd i", dd=WIN),
                            win_ap(mt[:], k * JW + 1, 1, 1),
                            win_ap(mt[:], k * JW, 0, 1),
                            Alu.max,
                        )
                    if 2 <= k:
                        layer(k - 2, range(NBA))

                # S = sum_k M via identity-matmul accumulation over k blocks
                sp = gps.tile([128, JW], dt.float32, tag="gm", name="sp")
                for k in range(K):
                    nc.tensor.matmul(
                        sp[:],
                        id_sb[:],
                        mt[:, k * JW : (k + 1) * JW],
                        start=(k == 0),
                        stop=(k == K - 1),
                    )
                nc.scalar.mul(sneg[:], sp[:], -0.5)
                nc.vector.tensor_tensor(
                    ssum[:].rearrange("p (dd i) -> p dd i", dd=WIN),
                    win_ap(sneg[:], 1, 1, 1),
                    win_ap(sneg[:], 0, 0, 1),
                    Alu.add,
                )
                k = K - 2
                nc.vector.tensor_tensor(
                    d[k][:].rearrange("p (dd i) -> p dd i", dd=WIN),
                    win_ap(mt[:], k * JW + 1, 1, 1),
                    win_ap(mt[:], k * JW, 0, 1),
                    Alu.max,
                )
                k = K - 1
                for h in range(2):
                    hw_ = WIN // 2
                    prow = list(mt[:].ap[0])
                    in0 = AP(mt[:].tensor, k * JW + 1 + hw_ * h,
                             [prow, [1, hw_], [1, IB]])
                    in1 = AP(mt[:].tensor, k * JW,
                             [prow, [0, hw_], [1, IB]])
                    nc.vector.tensor_tensor(
                        d[k][:, h * 2048 : (h + 1) * 2048].rearrange(
                            "p (dd i) -> p dd i", dd=hw_
                        ),
                        in0,
                        in1,
                        Alu.max,
                    )

            with tc.tile_pool(name="kpsB", bufs=NB - NBA, space="PSUM") as kpsB:
                for b in range(NBA, NB):
                    pts[b] = kpsB.tile([128, 512], dt.float32, tag="pt", name=f"pt{b}")
                for k in range(K - 2):
                    layer(k, range(NBA, NB))
                # seed: h += -(S_i+S_j)/2; group still open
                for b in range(NB):
                    nc.tensor.matmul(
                        pts[b][:],
                        id_sb[:],
                        ssum[:, b * 512 : (b + 1) * 512],
                        start=False,
                        stop=False,
                        skip_group_check=True,
                    )
                layer(K - 2, range(NB))

                def finish(banks):
                    for b in banks:
                        nc.tensor.matmul(
                            pts[b][:],
                            id_sb[:],
                            d[K - 1][:, b * 512 : (b + 1) * 512],
                            start=False,
                            stop=True,
                            skip_group_check=True,
                        )
                    for b in banks:
                        nc.scalar.activation(
                            band[:, b * 512 : (b + 1) * 512],
                            pts[b][:],
                            Act.Exp,
                            scale=-2.0,
                        )
                        nc.gpsimd.dma_start(
                            eout[:, b * 512 : (b + 1) * 512],
                            band[:, b * 512 : (b + 1) * 512],
                        )

                finish(range(4))
                finish(range(4, NB))

    nc.compile()
    return nc


def _prep_inputs(x, T):
    bf16 = ml_dtypes.bfloat16
    fp8 = ml_dtypes.float8_e4m3
    # W image: chunk k (k-major), col o; row p = f%128, col = k*FC*128 + fc*128 + o
    Wp = np.asarray(T, np.float32).transpose(2, 1, 0)  # (K, O, F)
    Wimg = (
        Wp.reshape(K, O, FC, 128).transpose(3, 0, 2, 1).reshape(128, -1)
    )  # (p, k*FC*O)
    Wimg = np.ascontiguousarray(Wimg).astype(fp8)
    xTf = np.asarray(x, np.float32).T  # (F, B)
    idm = np.eye(128, dtype=bf16)
    in_maps = []
    for b in range(NCORES):
        xl = np.roll(xTf, -IB * b, axis=1)[:, :JW]  # (F, 160)
        xi = np.ascontiguousarray(
            xl.reshape(FC, 128, JW).transpose(1, 0, 2).reshape(128, -1)
        ).astype(fp8)
        in_maps.append(
            {
                "w0": Wimg[:, : 4 * FC * 128],
                "w1": Wimg[:, 4 * FC * 128 :],
                "xt": xi,
                "idm": idm,
            }
        )
    return in_maps


def _assemble(x, results):
    c = np.zeros((B, O), np.float32)
    ar = np.arange(IB)
    for b in range(NCORES):
        E = results[b]["eb"].astype(np.float32).reshape(O, WIN, IB)  # (o, delta, i)
        rows = (IB * b + ar) % B
        c[rows] += E.sum(axis=1).T  # row part: sum over delta
        colsum = np.zeros((O, IB + WIN), np.float32)  # local j in [0, 160)
        for i in range(IB):
            colsum[:, i + 1 : i + 1 + WIN] += E[:, :, i]
        gj = (IB * b + np.arange(IB + WIN)) % B
        np.add.at(c, gj, colsum.T)
    return np.concatenate([np.asarray(x, np.float32), c], axis=1)


def _get_nc():
    if "nc" not in _cache:
        _cache["nc"] = _build()
    return _cache["nc"]


def kernel(x, T):
    from concourse.bass_utils import run_bass_kernel_spmd

    x = np.asarray(x)
    T = np.asarray(T)
    nc = _get_nc()
    res = run_bass_kernel_spmd(nc, _prep_inputs(x, T), list(range(NCORES)))
    return _assemble(x, res.results)


def run_traced(x, T, **kwargs):
    from concourse.bass_utils import run_bass_kernel_spmd

    x = np.asarray(x)
    T = np.asarray(T)
    nc = _get_nc()
    res = run_bass_kernel_spmd(
        nc, _prep_inputs(x, T), list(range(NCORES)), trace=True, **kwargs
    )
    return _assemble(x, res.results), res
